# revision 14
# baseline (speedup 1.0000x reference)
"""Causal self-attention (B=4, T=2048, C=1024, H=16, D=64) on 8 TRN2 NeuronCores.

Sharding: core c handles batch b = c//2 and head-group hg = c%2 (8 of 16 heads).
Per core: column-sharded QKV projection (only its heads' q/k/v columns, only its
batch's rows), full causal attention for its 8 heads, row-sharded output
projection producing a partial [T, C] result. Host sums the two head-group
partials per batch (the "all-reduce") and adds the bias correction term.

Math notes:
 - k-bias is dropped: softmax((q+bq)@(k+bk)^T) == softmax((q+bq)@k^T) because
   the (q+bq)@bk term is constant along the key axis.
 - v-bias and proj-bias are folded into a host-side correction: since softmax
   rows sum to 1, y = P@(V + 1 bv^T) = P@V + 1 bv^T, so the output correction
   is bv @ w_proj + b_proj added to every row.
 - Attention works in S^T layout ([keys, q]): softmax denominators come from a
   ones-column appended to V (row 64 of the PV accumulation), and the PV
   matmul V'.T @ P^T lands y^T in [d, q] layout which feeds the output
   projection directly as the stationary operand - no transposes anywhere.
 - Normalization (divide by softmax denominator, which varies along the free
   q axis) is done by DVE reciprocal of the denominator row + gpsimd
   partition-broadcast + elementwise multiply.
 - The q/k projection uses fp8 DoubleRow (2 contraction rows/cycle). V stays
   bf16: softmax output is spiky, so fp8 V quantization error (~6% per
   element) passes straight through to y and blows the error budget.
 - Output projection matmuls are interleaved into the attention stream
   (attention is exp-bound on the Scalar engine; the spare PE cycles there
   absorb most of the projection).
"""

import numpy as np
import ml_dtypes

B, T, C, H, D = 4, 2048, 1024, 16, 64
HC = 8            # heads per core
RC = T // 128     # 16 row chunks
QQ = T // 512     # 4 query super-blocks
BF16 = ml_dtypes.bfloat16
E4 = ml_dtypes.float8_e4m3fn

_COMPILED = {}


def _build_nc():
    from concourse import bacc
    import concourse.tile as tile
    from concourse import mybir

    bf16 = mybir.dt.bfloat16
    f32 = mybir.dt.float32
    fp8 = mybir.dt.float8e4
    EXP = mybir.ActivationFunctionType.Exp
    COPY = mybir.ActivationFunctionType.Copy
    ADD = mybir.AluOpType.add
    MULT = mybir.AluOpType.mult
    DR = mybir.MatmulPerfMode.DoubleRow

    nc = bacc.Bacc(None, target_bir_lowering=False)

    # x^T * 16 packed for per-512-row contiguous DMA: [p, r5, j, i, 512]
    # with contraction dim c = j*256 + i*128 + p, token t = r5*512 + tt.
    xT8 = nc.dram_tensor("xT8", [128, 4, 4, 2, 512], fp8, kind="ExternalInput")
    # x^T bf16 for the V projection: [p, r5, kc, 512], c = kc*128 + p.
    xT = nc.dram_tensor("xT", [128, 4, 8, 512], bf16, kind="ExternalInput")
    wqk8 = nc.dram_tensor("wqk8", [128, 4, 2, 8, 128], fp8, kind="ExternalInput")
    wv = nc.dram_tensor("wv", [128, 8, 512], bf16, kind="ExternalInput")
    bq = nc.dram_tensor("bq", [128, 4], f32, kind="ExternalInput")
    wp = nc.dram_tensor("wp", [128, 4, 1024], bf16, kind="ExternalInput")
    out = nc.dram_tensor("out", [T, C], bf16, kind="ExternalOutput")

    # Causal mask for the diagonal 128-key x 512-q blocks, variant r = kc % 4:
    # valid iff r*128 + k <= q. Applied multiplicatively to exp(S) in bf16.
    kk = np.arange(128)[:, None, None]
    rr = np.arange(4)[None, :, None]
    qq = np.arange(512)[None, None, :]
    mask_np = (rr * 128 + kk <= qq).astype(BF16)
    msk = nc.inline_tensor(mask_np, name="msk")

    with tile.TileContext(nc) as tc:
        with tc.tile_pool(name="singles", bufs=1) as singles:
            wqk_sb = singles.tile([128, 4, 2, 8, 128], fp8)
            wv_sb = singles.tile([128, 8, 512], bf16)
            bq_sb = singles.tile([128, 4], f32)
            wp_sb = singles.tile([128, 4, 1024], bf16)
            msk_sb = singles.tile([128, 4, 512], bf16)
            warm = singles.tile([8, 4], f32)
            # wqk8 on the SP ring, first xT8 chunk on the ACT ring (emitted
            # in the phase-1 pool below): both land ~3us in so the first
            # projection matmul starts early. Everything else queues behind.
            nc.sync.dma_start(wqk_sb[:], wqk8[:])
            nc.sync.dma_start(wv_sb[:], wv[:])
            nc.sync.dma_start(msk_sb[:], msk[:])
            nc.sync.dma_start(wp_sb[:], wp[:])

            # persistent activations
            qT_sb = singles.tile([128, 4, T], bf16)   # q^T, heads 2c,2c+1 in chunk c
            kT_sb = singles.tile([128, 4, T], bf16)
            v_sb = singles.tile([128, RC, HC, 65], bf16)  # V natural + ones col
            # y^T (d, q) per head pair: head 2pr at partitions 0-63,
            # head 2pr+1 at partitions 64-127. Stationary operand of the
            # output projection. One tile per pair so the framework tracks
            # write/read regions precisely (a single [128,4,T] tile makes
            # every projection read falsely depend on every normalize write).
            yT_sb = [singles.tile([128, T], bf16, name=f"yT{t}")
                     for t in range(4)]

            nc.vector.memset(v_sb[:, :, :, 64], 1.0)
            # warm up the gpsimd ucode (first op pays an IRAM load)
            nc.gpsimd.partition_broadcast(warm[:], bq_sb[0:1, :])

            # ---- Phase 1: QKV projection (q/k fp8 DoubleRow, v bf16) ----
            with tc.tile_pool(name="xt", bufs=2) as xp, \
                 tc.tile_pool(name="psA", bufs=4, space="PSUM") as psA:
                xts = []
                for r5 in range(4):           # 512-row chunks
                    xt8 = xp.tile([128, 4, 2, 512], fp8, name="xt8")
                    nc.scalar.dma_start(xt8[:], xT8[:, r5])
                    if r5 == 0:
                        nc.scalar.dma_start(bq_sb[:], bq[:])
                    xt = xp.tile([128, 8, 512], bf16, name="xtb")
                    nc.scalar.dma_start(xt[:], xT[:, r5])
                    xts.append((xt8, xt))
                for r5 in range(4):
                    sl = slice(r5 * 512, (r5 + 1) * 512)
                    xt8, xt = xts[r5]
                    for cc in range(8):       # qk column chunks (0-3 q, 4-7 k)
                        ps = psA.tile([128, 512], f32)
                        for j in range(4):    # fp8 DoubleRow: 2 kc per matmul
                            nc.tensor.matmul(ps[:], wqk_sb[:, j, :, cc, :],
                                             xt8[:, j, :, :], perf_mode=DR,
                                             start=(j == 0), stop=(j == 3))
                        if cc < 4:
                            # x scaled x16, w x64 on host: ps = 1024*(q+bq/1024),
                            # kT = 1024*k, so fold 0.125/1024^2 into q
                            nc.vector.tensor_scalar(
                                out=qT_sb[:, cc, sl], in0=ps[:],
                                scalar1=bq_sb[:, cc:cc + 1],
                                scalar2=0.125 / 1048576.0,
                                op0=ADD, op1=MULT)
                        else:
                            nc.scalar.activation(out=kT_sb[:, cc - 4, sl],
                                                 in_=ps[:], func=COPY)
                    for rs in range(4):       # v rows, 128 at a time
                        rc = r5 * 4 + rs
                        psv = psA.tile([128, 512], f32)
                        for kc in range(8):
                            nc.tensor.matmul(
                                psv[:], xt[:, kc, rs * 128:(rs + 1) * 128],
                                wv_sb[:, kc, :],
                                start=(kc == 0), stop=(kc == 7))
                        nc.scalar.activation(
                            out=v_sb[:, rc, :, 0:64],
                            in_=psv[:].rearrange("p (h d) -> p h d", h=HC),
                            func=COPY)

            # ---- Phase 2+3: attention with interleaved output projection ----
            # S^T = K^T.T @ Q^T per 128-key chunk (two heads packed in the PE
            # array via partition offsets); exp on ACT; causal mask multiplied
            # into exp(S) in bf16; wide PV: y^T[65,512] += V'.T @ expS with V'
            # stationary. Row 64 of y^T is the softmax denominator; normalize
            # via gpsimd broadcast + DVE reciprocal/multiply straight into
            # yT_sb. Output projection rows are injected one rc per attention
            # block (attention is ACT-bound; PE has spare cycles).
            with tc.tile_pool(name="att", bufs=6) as ap, \
                 tc.tile_pool(name="stg", bufs=6) as stg, \
                 tc.tile_pool(name="nrm", bufs=2) as nrm, \
                 tc.tile_pool(name="outp", bufs=2) as op_, \
                 tc.tile_pool(name="psS", bufs=2, space="PSUM") as psS, \
                 tc.tile_pool(name="psY", bufs=1, space="PSUM") as psY, \
                 tc.tile_pool(name="psO", bufs=1, space="PSUM") as psO:

                def emit_stage(pr, q5, psyt):
                    # free the PSUM accumulator quickly; the rest of the
                    # normalize chain runs later off this SBUF copy
                    stag = stg.tile([65, 2, 512], f32, name="stag")
                    nc.vector.tensor_copy(out=stag[:], in_=psyt[:])
                    return (pr, q5, stag)

                def emit_normalize(pend):
                    pr, q5, stag = pend
                    qsl = slice(q5 * 512, (q5 + 1) * 512)
                    # denominator row (partition 64) -> partition 0,
                    # reciprocal there (cheap custom DVE op), broadcast
                    rrow = nrm.tile([1, 2, 512], f32, name="rrow")
                    nc.scalar.dma_start(rrow[:], stag[64:65, :, :])
                    rrec = nrm.tile([1, 2, 512], f32, name="rrec")
                    nc.vector.reciprocal_approx_fast(out=rrec[:], in_=rrow[:])
                    recB = nrm.tile([64, 2, 512], f32, name="recB")
                    nc.gpsimd.partition_broadcast(recB[:], rrec[:])
                    nc.vector.tensor_mul(out=yT_sb[pr][0:64, qsl],
                                         in0=stag[0:64, 0, :],
                                         in1=recB[:, 0, :])
                    h1s = nrm.tile([64, 512], bf16, name="h1s")
                    nc.vector.tensor_mul(out=h1s[:], in0=stag[0:64, 1, :],
                                         in1=recB[:, 1, :])
                    # odd head -> partitions 64-127 of the pair chunk
                    nc.sync.dma_start(yT_sb[pr][64:128, qsl], h1s[:])

                def emit_proj(rc):
                    # per-oh halves so the cast/DMA of one PSUM bank overlaps
                    # the matmuls of the other (psO is single-buffered)
                    pso = psO.tile([128, 2, 512], f32)
                    for oh in range(2):
                        for t in range(4):
                            nc.tensor.matmul(
                                pso[:, oh, :],
                                yT_sb[t][:, rc * 128:(rc + 1) * 128],
                                wp_sb[:, t, oh * 512:(oh + 1) * 512],
                                start=(t == 0), stop=(t == 3),
                                skip_group_check=True)
                        osb = op_.tile([128, 512], bf16)
                        nc.vector.tensor_copy(out=osb[:], in_=pso[:, oh, :])
                        eng = nc.sync if (rc + oh) % 2 == 0 else nc.scalar
                        eng.dma_start(
                            out[rc * 128:(rc + 1) * 128,
                                oh * 512:(oh + 1) * 512], osb[:])

                def emit_S(pr, q5, kc):
                    diag = (kc // 4 == q5)
                    qof = (kc % 4) * 128 if diag else 0
                    pss = psS.tile([128, 2, 512], f32, name="pss")
                    for i in range(2):   # head in pair, packed in PE rows
                        po = i * 64
                        nc.tensor.matmul(
                            pss[:, i, qof:],
                            kT_sb[po:po + 64, pr, kc * 128:(kc + 1) * 128],
                            qT_sb[po:po + 64, pr,
                                  q5 * 512 + qof:(q5 + 1) * 512],
                            start=True, stop=True)
                    return pss, qof, diag

                norm_queue = []   # staged (pr, q5, stag) awaiting normalize
                normed = set()    # q5 rounds fully written into yT_sb
                proj_queue = []   # rc bands awaiting output projection
                for q5 in range(QQ):
                    for pr in range(4):       # head pair 2pr, 2pr+1
                        nkc = 4 * (q5 + 1)
                        psyt = psY.tile([65, 2, 512], f32, name="psyt")
                        # software pipeline: keep 2 S blocks in flight so the
                        # PE has work while ACT runs exp, and normalize/proj
                        # of earlier blocks slots into the exp-bound stream.
                        window = [emit_S(pr, q5, kc) for kc in range(2)]
                        if proj_queue:
                            emit_proj(proj_queue.pop(0))
                        for _ in range(2):
                            if norm_queue:
                                pend = norm_queue.pop(0)
                                emit_normalize(pend)
                                if pend[0] == 3:
                                    normed.add(pend[1])
                                    proj_queue.extend(
                                        range(4 * pend[1], 4 * pend[1] + 4))
                        for kc in range(nkc):
                            pss, qof, diag = window[0]
                            exps = ap.tile([128, 2, 512], bf16)
                            nc.scalar.activation(exps[:, :, qof:],
                                                 pss[:, :, qof:], EXP)
                            if diag:
                                r = kc % 4
                                for i in range(2):
                                    nc.vector.tensor_mul(
                                        out=exps[:, i, qof:qof + 128],
                                        in0=exps[:, i, qof:qof + 128],
                                        in1=msk_sb[:, r, qof:qof + 128])
                            window.pop(0)
                            if kc + 2 < nkc:
                                window.append(emit_S(pr, q5, kc + 2))
                            for i in range(2):
                                nc.tensor.matmul(
                                    psyt[:, i, qof:], v_sb[:, kc, 2 * pr + i, :],
                                    exps[:, i, qof:],
                                    start=(kc == 0), stop=(kc == nkc - 1))
                        norm_queue.append(emit_stage(pr, q5, psyt))
                for pend in norm_queue:
                    emit_normalize(pend)
                proj_queue.extend(r for r in range(RC)
                                  if r not in proj_queue and r // 4 not in normed)
                for rc in proj_queue:
                    emit_proj(rc)

    nc.compile()
    return nc


def _prep_core_inputs(x, w_attn, b_attn, w_proj, c):
    b, hg = c // 2, c % 2
    xb = np.ascontiguousarray(x[b])                       # [T, C]
    xT8 = np.ascontiguousarray(np.clip(xb.T * 16.0, -240, 240)
                               .reshape(4, 2, 128, 4, 512)
                               .transpose(2, 3, 0, 1, 4)).astype(E4)
    xT = np.ascontiguousarray(
        xb.T.reshape(8, 128, 4, 512).transpose(1, 2, 0, 3)).astype(BF16)
    wq = w_attn[:, hg * 512:(hg + 1) * 512]
    wk = w_attn[:, C + hg * 512:C + (hg + 1) * 512]
    wqk = np.concatenate([wq, wk], axis=1)                # [C, 1024]
    wqk8 = np.ascontiguousarray(np.clip(wqk * 64.0, -240, 240)
                                .reshape(4, 2, 128, 8, 128)
                                .transpose(2, 0, 1, 3, 4)).astype(E4)
    wvc = w_attn[:, 2 * C + hg * 512:2 * C + (hg + 1) * 512]
    wvc = np.ascontiguousarray(
        wvc.reshape(8, 128, 512).transpose(1, 0, 2)).astype(BF16)
    bqv = np.ascontiguousarray(
        b_attn[hg * 512:(hg + 1) * 512].reshape(4, 128).T
        * 1024.0).astype(np.float32)
    wpc = w_proj[hg * 512:(hg + 1) * 512, :]
    wpc = np.ascontiguousarray(
        wpc.reshape(4, 128, 1024).transpose(1, 0, 2)).astype(BF16)
    return {"xT8": xT8, "xT": xT, "wqk8": wqk8, "wv": wvc, "bq": bqv,
            "wp": wpc}


def _run(nc, in_maps, **kwargs):
    from concourse.bass_utils import run_bass_kernel_spmd
    return run_bass_kernel_spmd(nc, in_maps, core_ids=list(range(8)), **kwargs)


def kernel(x, w_attn, b_attn, w_proj, b_proj, _trace=False):
    x = np.asarray(x, dtype=np.float32)
    w_attn = np.asarray(w_attn, dtype=np.float32)
    b_attn = np.asarray(b_attn, dtype=np.float32)
    w_proj = np.asarray(w_proj, dtype=np.float32)
    b_proj = np.asarray(b_proj, dtype=np.float32)

    if "nc" not in _COMPILED:
        _COMPILED["nc"] = _build_nc()
    nc = _COMPILED["nc"]

    in_maps = [_prep_core_inputs(x, w_attn, b_attn, w_proj, c) for c in range(8)]
    kwargs = {"trace": True} if _trace else {}
    res = _run(nc, in_maps, **kwargs)
    _COMPILED["last_result"] = res

    corr = b_attn[2 * C:].astype(np.float32) @ w_proj + b_proj
    out = np.empty((B, T, C), np.float32)
    for b in range(B):
        out[b] = (res.results[2 * b]["out"].astype(np.float32)
                  + res.results[2 * b + 1]["out"].astype(np.float32))
        out[b] += corr[None, :]
    return out


# revision 21
# speedup vs baseline: 1.0737x; 1.0737x over previous
"""Causal self-attention (B=4, T=2048, C=1024, H=16, D=64) on 8 TRN2 NeuronCores.

Sharding: core c handles batch b = c//2 and head-group hg = c%2 (8 of 16 heads).
Per core: column-sharded QKV projection (only its heads' q/k/v columns, only its
batch's rows), full causal attention for its 8 heads, row-sharded output
projection producing a partial [T, C] result. Host sums the two head-group
partials per batch (the "all-reduce") and adds the bias correction term.

Math notes:
 - k-bias is dropped: softmax((q+bq)@(k+bk)^T) == softmax((q+bq)@k^T) because
   the (q+bq)@bk term is constant along the key axis.
 - v-bias and proj-bias are folded into a host-side correction: since softmax
   rows sum to 1, y = P@(V + 1 bv^T) = P@V + 1 bv^T, so the output correction
   is bv @ w_proj + b_proj added to every row.
 - Attention works in S^T layout ([keys, q]): softmax denominators come from a
   ones-column appended to V (row 64 of the PV accumulation), and the PV
   matmul V'.T @ P^T lands y^T in [d, q] layout which feeds the output
   projection directly as the stationary operand - no transposes anywhere.
 - Normalization (divide by softmax denominator, which varies along the free
   q axis) is done by DVE reciprocal of the denominator row + gpsimd
   partition-broadcast + elementwise multiply.
 - The q/k projection uses fp8 DoubleRow (2 contraction rows/cycle). V stays
   bf16: softmax output is spiky, so fp8 V quantization error (~6% per
   element) passes straight through to y and blows the error budget.
 - Output projection matmuls are interleaved into the attention stream
   (attention is exp-bound on the Scalar engine; the spare PE cycles there
   absorb most of the projection).
"""

import numpy as np
import ml_dtypes

B, T, C, H, D = 4, 2048, 1024, 16, 64
HC = 8            # heads per core
RC = T // 128     # 16 row chunks
QQ = T // 512     # 4 query super-blocks
BF16 = ml_dtypes.bfloat16
E4 = ml_dtypes.float8_e4m3fn

_COMPILED = {}


def _build_nc():
    from concourse import bacc
    import concourse.tile as tile
    from concourse import mybir

    bf16 = mybir.dt.bfloat16
    f32 = mybir.dt.float32
    fp8 = mybir.dt.float8e4
    EXP = mybir.ActivationFunctionType.Exp
    COPY = mybir.ActivationFunctionType.Copy
    ADD = mybir.AluOpType.add
    MULT = mybir.AluOpType.mult
    DR = mybir.MatmulPerfMode.DoubleRow

    nc = bacc.Bacc(None, target_bir_lowering=False)

    # x^T * 16 packed for per-512-row contiguous DMA: [p, r5, j, i, 512]
    # with contraction dim c = j*256 + i*128 + p, token t = r5*512 + tt.
    xT8 = nc.dram_tensor("xT8", [128, 4, 4, 2, 512], fp8, kind="ExternalInput")
    # x^T bf16 for the V projection: [p, r5, kc, 512], c = kc*128 + p.
    xT = nc.dram_tensor("xT", [128, 4, 8, 512], bf16, kind="ExternalInput")
    wqk8 = nc.dram_tensor("wqk8", [128, 4, 2, 8, 128], fp8, kind="ExternalInput")
    wv = nc.dram_tensor("wv", [128, 8, 512], bf16, kind="ExternalInput")
    bq = nc.dram_tensor("bq", [128, 4], f32, kind="ExternalInput")
    wp = nc.dram_tensor("wp", [128, 4, 1024], bf16, kind="ExternalInput")
    out = nc.dram_tensor("out", [T, C], bf16, kind="ExternalOutput")

    # Causal mask for the diagonal 128-key x 512-q blocks, variant r = kc % 4:
    # valid iff r*128 + k <= q. Applied multiplicatively to exp(S) in bf16.
    kk = np.arange(128)[:, None, None]
    rr = np.arange(4)[None, :, None]
    qq = np.arange(512)[None, None, :]
    mask_np = (rr * 128 + kk <= qq).astype(BF16)
    msk = nc.inline_tensor(mask_np, name="msk")

    with tile.TileContext(nc) as tc:
        with tc.tile_pool(name="singles", bufs=1) as singles:
            wqk_sb = singles.tile([128, 4, 2, 8, 128], fp8)
            wv_sb = singles.tile([128, 8, 512], bf16)
            bq_sb = singles.tile([128, 4], f32)
            wp_sb = singles.tile([128, 4, 1024], bf16)
            msk_sb = singles.tile([128, 4, 512], bf16)
            warm = singles.tile([8, 4], f32)
            # wqk8 on the SP ring, first xT8 chunk on the ACT ring (emitted
            # in the phase-1 pool below): both land ~3us in so the first
            # projection matmul starts early. Everything else queues behind.
            nc.sync.dma_start(wqk_sb[:], wqk8[:])
            nc.sync.dma_start(wv_sb[:], wv[:])
            nc.sync.dma_start(msk_sb[:], msk[:])
            nc.sync.dma_start(wp_sb[:], wp[:])

            # persistent activations
            qT_sb = singles.tile([128, 4, T], bf16)   # q^T, heads 2c,2c+1 in chunk c
            kT_sb = singles.tile([128, 4, T], bf16)
            v_sb = singles.tile([128, RC, HC, 65], bf16)  # V natural + ones col
            # y^T (d, q) per head pair: head 2pr at partitions 0-63,
            # head 2pr+1 at partitions 64-127. Stationary operand of the
            # output projection. One tile per pair so the framework tracks
            # write/read regions precisely (a single [128,4,T] tile makes
            # every projection read falsely depend on every normalize write).
            yT_sb = [singles.tile([128, T], bf16, name=f"yT{t}")
                     for t in range(4)]

            nc.vector.memset(v_sb[:, :, :, 64], 1.0)
            # warm up the gpsimd ucode (first op pays an IRAM load)
            nc.gpsimd.partition_broadcast(warm[:], bq_sb[0:1, :])

            # ---- Phase 1: QKV projection (q/k fp8 DoubleRow, v bf16) ----
            with tc.tile_pool(name="xt", bufs=2) as xp, \
                 tc.tile_pool(name="psA", bufs=4, space="PSUM") as psA:
                xts = []
                for r5 in range(4):           # 512-row chunks
                    xt8 = xp.tile([128, 4, 2, 512], fp8, name="xt8")
                    nc.scalar.dma_start(xt8[:], xT8[:, r5])
                    if r5 == 0:
                        nc.scalar.dma_start(bq_sb[:], bq[:])
                    xt = xp.tile([128, 8, 512], bf16, name="xtb")
                    nc.scalar.dma_start(xt[:], xT[:, r5])
                    xts.append((xt8, xt))
                for r5 in range(4):
                    sl = slice(r5 * 512, (r5 + 1) * 512)
                    xt8, xt = xts[r5]
                    for cc in range(8):       # qk column chunks (0-3 q, 4-7 k)
                        ps = psA.tile([128, 512], f32)
                        for j in range(4):    # fp8 DoubleRow: 2 kc per matmul
                            nc.tensor.matmul(ps[:], wqk_sb[:, j, :, cc, :],
                                             xt8[:, j, :, :], perf_mode=DR,
                                             start=(j == 0), stop=(j == 3))
                        if cc < 4:
                            # x scaled x16, w x64 on host: ps = 1024*(q+bq/1024),
                            # kT = 1024*k, so fold 0.125/1024^2 into q
                            nc.vector.tensor_scalar(
                                out=qT_sb[:, cc, sl], in0=ps[:],
                                scalar1=bq_sb[:, cc:cc + 1],
                                scalar2=0.125 / 1048576.0,
                                op0=ADD, op1=MULT)
                        else:
                            nc.scalar.activation(out=kT_sb[:, cc - 4, sl],
                                                 in_=ps[:], func=COPY)
                    for rs in range(4):       # v rows, 128 at a time
                        rc = r5 * 4 + rs
                        psv = psA.tile([128, 512], f32)
                        for kc in range(8):
                            nc.tensor.matmul(
                                psv[:], xt[:, kc, rs * 128:(rs + 1) * 128],
                                wv_sb[:, kc, :],
                                start=(kc == 0), stop=(kc == 7))
                        nc.scalar.activation(
                            out=v_sb[:, rc, :, 0:64],
                            in_=psv[:].rearrange("p (h d) -> p h d", h=HC),
                            func=COPY)

            # ---- Phase 2+3: attention with interleaved output projection ----
            # S^T = K^T.T @ Q^T per 128-key chunk (two heads packed in the PE
            # array via partition offsets); exp on ACT; causal mask multiplied
            # into exp(S) in bf16; wide PV: y^T[65,512] += V'.T @ expS with V'
            # stationary. Row 64 of y^T is the softmax denominator; normalize
            # via gpsimd broadcast + DVE reciprocal/multiply straight into
            # yT_sb. Output projection rows are injected one rc per attention
            # block (attention is ACT-bound; PE has spare cycles).
            with tc.tile_pool(name="att", bufs=6) as ap, \
                 tc.tile_pool(name="stg", bufs=6) as stg, \
                 tc.tile_pool(name="nrm", bufs=2) as nrm, \
                 tc.tile_pool(name="outp", bufs=3) as op_:
                psS = psY = psO = None

                def emit_stage(pr, q5, psyt):
                    # free the PSUM accumulator quickly; the rest of the
                    # normalize chain runs later off this SBUF copy
                    stag = stg.tile([65, 2, 512], f32, name="stag")
                    nc.vector.tensor_copy(out=stag[:], in_=psyt[:])
                    return (pr, q5, stag)

                def emit_normalize(pend):
                    pr, q5, stag = pend
                    qsl = slice(q5 * 512, (q5 + 1) * 512)
                    # denominator row (partition 64) -> partition 0,
                    # reciprocal there (cheap custom DVE op), broadcast
                    rrow = nrm.tile([1, 2, 512], f32, name="rrow")
                    nc.scalar.dma_start(rrow[:], stag[64:65, :, :])
                    rrec = nrm.tile([1, 2, 512], f32, name="rrec")
                    nc.vector.reciprocal_approx_fast(out=rrec[:], in_=rrow[:])
                    recB = nrm.tile([64, 2, 512], f32, name="recB")
                    nc.gpsimd.partition_broadcast(recB[:], rrec[:])
                    nc.vector.tensor_mul(out=yT_sb[pr][0:64, qsl],
                                         in0=stag[0:64, 0, :],
                                         in1=recB[:, 0, :])
                    h1s = nrm.tile([64, 512], bf16, name="h1s")
                    nc.vector.tensor_mul(out=h1s[:], in0=stag[0:64, 1, :],
                                         in1=recB[:, 1, :])
                    # odd head -> partitions 64-127 of the pair chunk
                    nc.sync.dma_start(yT_sb[pr][64:128, qsl], h1s[:])

                def emit_proj(rc, pool=None):
                    pso = (pool or psO).tile([128, 2, 512], f32, name="pso")
                    for t in range(4):
                        for oh in range(2):
                            nc.tensor.matmul(
                                pso[:, oh, :],
                                yT_sb[t][:, rc * 128:(rc + 1) * 128],
                                wp_sb[:, t, oh * 512:(oh + 1) * 512],
                                start=(t == 0), stop=(t == 3),
                                skip_group_check=True)
                    osb = op_.tile([128, 1024], bf16)
                    nc.vector.tensor_copy(
                        out=osb[:].rearrange("p (a b) -> p a b", a=2), in_=pso[:])
                    eng = nc.sync if rc % 2 == 0 else nc.scalar
                    eng.dma_start(out[rc * 128:(rc + 1) * 128, :], osb[:])

                def emit_S(pr, q5, kc):
                    diag = (kc // 4 == q5)
                    qof = (kc % 4) * 128 if diag else 0
                    pss = psS.tile([128, 2, 512], f32, name="pss")
                    for i in range(2):   # head in pair, packed in PE rows
                        po = i * 64
                        nc.tensor.matmul(
                            pss[:, i, qof:],
                            kT_sb[po:po + 64, pr, kc * 128:(kc + 1) * 128],
                            qT_sb[po:po + 64, pr,
                                  q5 * 512 + qof:(q5 + 1) * 512],
                            start=True, stop=True)
                    return pss, qof, diag

                norm_queue = []   # staged (pr, q5, stag) awaiting normalize
                normed = set()    # q5 rounds fully written into yT_sb
                proj_queue = []   # rc bands awaiting output projection
                att_pools = tc.tile_pool(name="psS", bufs=2, space="PSUM")
                psS = att_pools.__enter__()
                psY_cm = tc.tile_pool(name="psY", bufs=1, space="PSUM")
                psY = psY_cm.__enter__()
                psO_cm = tc.tile_pool(name="psO", bufs=1, space="PSUM")
                psO = psO_cm.__enter__()
                for q5 in range(QQ):
                    for pr in range(4):       # head pair 2pr, 2pr+1
                        nkc = 4 * (q5 + 1)
                        psyt = psY.tile([65, 2, 512], f32, name="psyt")
                        # software pipeline: keep 2 S blocks in flight so the
                        # PE has work while ACT runs exp, and normalize/proj
                        # of earlier blocks slots into the exp-bound stream.
                        window = [emit_S(pr, q5, kc) for kc in range(2)]
                        if proj_queue:
                            emit_proj(proj_queue.pop(0))
                        for _ in range(2):
                            if norm_queue:
                                pend = norm_queue.pop(0)
                                emit_normalize(pend)
                                if pend[0] == 3:
                                    normed.add(pend[1])
                                    proj_queue.extend(
                                        range(4 * pend[1], 4 * pend[1] + 4))
                        for kc in range(nkc):
                            pss, qof, diag = window[0]
                            exps = ap.tile([128, 2, 512], bf16)
                            nc.scalar.activation(exps[:, :, qof:],
                                                 pss[:, :, qof:], EXP)
                            if diag:
                                r = kc % 4
                                for i in range(2):
                                    nc.vector.tensor_mul(
                                        out=exps[:, i, qof:qof + 128],
                                        in0=exps[:, i, qof:qof + 128],
                                        in1=msk_sb[:, r, qof:qof + 128])
                            window.pop(0)
                            if kc + 2 < nkc:
                                window.append(emit_S(pr, q5, kc + 2))
                            for i in range(2):
                                nc.tensor.matmul(
                                    psyt[:, i, qof:], v_sb[:, kc, 2 * pr + i, :],
                                    exps[:, i, qof:],
                                    start=(kc == 0), stop=(kc == nkc - 1))
                        norm_queue.append(emit_stage(pr, q5, psyt))
                # Tail: emit already-satisfiable projections BEFORE the final
                # normalize chain. DMA writes are tracked per-tile, so a proj
                # emitted after the last h1 DMA would falsely wait on it.
                for rc in proj_queue:
                    emit_proj(rc)
                late = [r for r in range(RC) if r // 4 not in normed]
                for pend in norm_queue:
                    emit_normalize(pend)
                # Close the attention PSUM pools, then run the remaining
                # projections of the last q5 round triple-buffered so casts
                # overlap matmuls and the PE stays clocked up.
                psO_cm.__exit__(None, None, None)
                psY_cm.__exit__(None, None, None)
                att_pools.__exit__(None, None, None)
                with tc.tile_pool(name="psT", bufs=3, space="PSUM") as psT:
                    for rc in late:
                        emit_proj(rc, pool=psT)

    nc.compile()
    return nc


def _prep_core_inputs(x, w_attn, b_attn, w_proj, c):
    b, hg = c // 2, c % 2
    xb = np.ascontiguousarray(x[b])                       # [T, C]
    xT8 = np.ascontiguousarray(np.clip(xb.T * 16.0, -240, 240)
                               .reshape(4, 2, 128, 4, 512)
                               .transpose(2, 3, 0, 1, 4)).astype(E4)
    xT = np.ascontiguousarray(
        xb.T.reshape(8, 128, 4, 512).transpose(1, 2, 0, 3)).astype(BF16)
    wq = w_attn[:, hg * 512:(hg + 1) * 512]
    wk = w_attn[:, C + hg * 512:C + (hg + 1) * 512]
    wqk = np.concatenate([wq, wk], axis=1)                # [C, 1024]
    wqk8 = np.ascontiguousarray(np.clip(wqk * 64.0, -240, 240)
                                .reshape(4, 2, 128, 8, 128)
                                .transpose(2, 0, 1, 3, 4)).astype(E4)
    wvc = w_attn[:, 2 * C + hg * 512:2 * C + (hg + 1) * 512]
    wvc = np.ascontiguousarray(
        wvc.reshape(8, 128, 512).transpose(1, 0, 2)).astype(BF16)
    bqv = np.ascontiguousarray(
        b_attn[hg * 512:(hg + 1) * 512].reshape(4, 128).T
        * 1024.0).astype(np.float32)
    wpc = w_proj[hg * 512:(hg + 1) * 512, :]
    wpc = np.ascontiguousarray(
        wpc.reshape(4, 128, 1024).transpose(1, 0, 2)).astype(BF16)
    return {"xT8": xT8, "xT": xT, "wqk8": wqk8, "wv": wvc, "bq": bqv,
            "wp": wpc}


def _run(nc, in_maps, **kwargs):
    from concourse.bass_utils import run_bass_kernel_spmd
    return run_bass_kernel_spmd(nc, in_maps, core_ids=list(range(8)), **kwargs)


def kernel(x, w_attn, b_attn, w_proj, b_proj, _trace=False):
    x = np.asarray(x, dtype=np.float32)
    w_attn = np.asarray(w_attn, dtype=np.float32)
    b_attn = np.asarray(b_attn, dtype=np.float32)
    w_proj = np.asarray(w_proj, dtype=np.float32)
    b_proj = np.asarray(b_proj, dtype=np.float32)

    if "nc" not in _COMPILED:
        _COMPILED["nc"] = _build_nc()
    nc = _COMPILED["nc"]

    in_maps = [_prep_core_inputs(x, w_attn, b_attn, w_proj, c) for c in range(8)]
    kwargs = {"trace": True} if _trace else {}
    res = _run(nc, in_maps, **kwargs)
    _COMPILED["last_result"] = res

    corr = b_attn[2 * C:].astype(np.float32) @ w_proj + b_proj
    out = np.empty((B, T, C), np.float32)
    for b in range(B):
        out[b] = (res.results[2 * b]["out"].astype(np.float32)
                  + res.results[2 * b + 1]["out"].astype(np.float32))
        out[b] += corr[None, :]
    return out
